# revision 8
# baseline (speedup 1.0000x reference)
"""Trainium2 Bass kernel for DecoderAttnRNN (LSTM + attention decoder).

Sharding: data-parallel over batch B=64 across 8 cores (8 batches/core).
Each core:
  phase 0: gather embeddings (indirect DMA), transpose, precompute x@W_ih.T+bias
  phase 1: 72-step LSTM recurrence in transposed layout (features on
           partitions, batch on free dim)
  phase 2: attention for all (b,t) at once (ctx does not feed the recurrence)
  phase 3: big output projection (576 x 512) @ (512 x 32000) in bf16,
           streaming W_lin^T from DRAM, b_lin added via K=1 broadcast matmuls
"""

import numpy as np
import ml_dtypes

import concourse.bass as bass
import concourse.mybir as mybir
import concourse.tile as tile
from concourse import bacc
from concourse.bass_utils import run_bass_kernel_spmd
from concourse.masks import make_identity

B, T, S, E, H, V = 64, 72, 72, 128, 256, 32000
NCORES = 8
BL = B // NCORES          # 8 batches per core
BT = BL * T               # 576 (t-major: flat index = t*BL + b)
G4H = 4 * H               # 1024
NCH = G4H // 128          # 8 gate chunks of 128
NC_N = 500                # psum n-chunk for phase 3 (4 per group)
NGC = 4 * NC_N            # 2000 cols per n-group
NGN = V // NGC            # 16 groups

f32 = mybir.dt.float32
bf16 = mybir.dt.bfloat16
i32 = mybir.dt.int32

M_TILES = [(0, 128), (128, 128), (256, 128), (384, 128), (512, 64)]

_CACHE = {}


def _build():
    nc = bacc.Bacc(None, target_bir_lowering=False)

    tok_d = nc.declare_dram_parameter("tok", [BT, 1], i32, isOutput=False)
    emb_d = nc.declare_dram_parameter("emb", [V, E], f32, isOutput=False)
    enc_d = nc.declare_dram_parameter("enc", [S, BL, H], bf16, isOutput=False)
    encT_d = nc.declare_dram_parameter("encT", [2, 128, BL, S], bf16, isOutput=False)
    h0T_d = nc.declare_dram_parameter("h0T", [128, 2, BL], f32, isOutput=False)
    c0T_d = nc.declare_dram_parameter("c0T", [128, 2, BL], f32, isOutput=False)
    lens_d = nc.declare_dram_parameter("lens", [BL], i32, isOutput=False)
    biasT_d = nc.declare_dram_parameter("biasT", [128, NCH], f32, isOutput=False)
    wihT_d = nc.declare_dram_parameter("wihT", [E, G4H], bf16, isOutput=False)
    whhT_d = nc.declare_dram_parameter("whhT", [2, 128, G4H], bf16, isOutput=False)
    wlinT_d = nc.declare_dram_parameter("wlinT", [4, 128, V], bf16, isOutput=False)
    blin_d = nc.declare_dram_parameter("blin", [1, V], bf16, isOutput=False)
    out_d = nc.declare_dram_parameter("logits", [BT, V], f32, isOutput=True)

    with tile.TileContext(nc) as tc:
        with tc.tile_pool(name="persist", bufs=1) as pp:
            # ---- persistent tiles ----
            wih_sb = pp.tile([128, G4H], bf16)
            nc.sync.dma_start(out=wih_sb[:], in_=wihT_d[:])
            whh_sb = pp.tile([128, 2, G4H], bf16)
            for k in range(2):
                nc.sync.dma_start(out=whh_sb[:, k, :], in_=whhT_d[k])
            biasT_sb = pp.tile([128, NCH], f32)
            nc.sync.dma_start(out=biasT_sb[:], in_=biasT_d[:])
            encT_sb = pp.tile([128, 2, BL, S], bf16)
            for k in range(2):
                nc.sync.dma_start(out=encT_sb[:, k], in_=encT_d[k])
            enc_sb = pp.tile([S, BL, H], bf16)
            nc.sync.dma_start(out=enc_sb[:], in_=enc_d[:])

            xwT = pp.tile([128, NCH, T, BL], f32)      # x@W_ih.T + bias, (t,b) cols
            z01 = pp.tile([128, 2, BL, T], bf16)       # h features (k-tiles 0,1)
            z23 = pp.tile([128, 2, BL, T], bf16)       # ctx features (k-tiles 2,3)
            x_allT = pp.tile([128, BT], bf16)          # embeddings^T, (t,b) cols

            cT = pp.tile([128, 2, BL], f32)
            nc.sync.dma_start(out=cT[:], in_=c0T_d[:])
            h0f = pp.tile([128, 2, BL], f32)
            nc.sync.dma_start(out=h0f[:], in_=h0T_d[:])
            h_init = pp.tile([128, 2, BL], bf16)
            nc.vector.tensor_copy(out=h_init[:], in_=h0f[:])

            ident = pp.tile([128, 128], f32)
            make_identity(nc, ident[:])
            ones_col = pp.tile([S, 1], bf16)
            nc.vector.memset(ones_col[:], 1.0)
            ones_row_f = pp.tile([1, 128], f32)
            nc.vector.memset(ones_row_f[:], 1.0)
            ones_row_bf = pp.tile([1, 128], bf16)
            nc.vector.memset(ones_row_bf[:], 1.0)

            # ---- attention mask: mask01[s, b] = 1.0 if s < len_b else 0.0 ----
            lens_i = pp.tile([S, BL], i32)
            lens_bcast = bass.AP(tensor=lens_d, offset=0, ap=[[0, S], [1, BL]])
            nc.sync.dma_start(out=lens_i[:], in_=lens_bcast)
            lens_f = pp.tile([S, BL], f32)
            nc.vector.tensor_copy(out=lens_f[:], in_=lens_i[:])
            iota_i = pp.tile([S, 1], i32)
            nc.gpsimd.iota(iota_i[:], [[1, 1]], base=0, channel_multiplier=1)
            iota_f = pp.tile([S, 1], f32)
            nc.vector.tensor_copy(out=iota_f[:], in_=iota_i[:])
            mask01 = pp.tile([S, BL], f32)
            nc.vector.tensor_scalar(
                out=mask01[:], in0=lens_f[:], scalar1=iota_f[:], scalar2=None,
                op0=mybir.AluOpType.is_gt,
            )

            # ---- phase 0: embedding gather + transpose ----
            with (
                tc.tile_pool(name="p0", bufs=2) as wp,
                tc.tile_pool(name="p0ps", bufs=2, space="PSUM") as psp,
            ):
                for j in range(5):
                    n = 128 if j < 4 else BT - 4 * 128
                    tok_t = wp.tile([128, 1], i32, tag="tok")
                    nc.sync.dma_start(
                        out=tok_t[:n], in_=tok_d[j * 128 : j * 128 + n]
                    )
                    x_t = wp.tile([128, E], f32, tag="x")
                    nc.gpsimd.indirect_dma_start(
                        out=x_t[:n],
                        out_offset=None,
                        in_=emb_d[:],
                        in_offset=bass.IndirectOffsetOnAxis(ap=tok_t[:n, :1], axis=0),
                    )
                    ps_t = psp.tile([128, 128], f32, tag="pst")
                    nc.tensor.transpose(
                        out=ps_t[:, :n], in_=x_t[:n], identity=ident[:n, :n]
                    )
                    nc.vector.tensor_copy(
                        out=x_allT[:, j * 128 : j * 128 + n], in_=ps_t[:, :n]
                    )

                # xW precompute: xwT[:, c, t, b] = (x @ W_ih.T)[tb, c*128:...] + bias
                for c in range(NCH):
                    ps_xw = psp.tile([128, BT], f32, tag="psxw")
                    for n0, nn in [(0, 512), (512, BT - 512)]:
                        nc.tensor.matmul(
                            ps_xw[:, n0 : n0 + nn],
                            wih_sb[:, c * 128 : (c + 1) * 128],
                            x_allT[:, n0 : n0 + nn],
                            start=True,
                            stop=True,
                        )
                    nc.vector.tensor_scalar(
                        out=xwT[:, c].rearrange("p t b -> p (t b)"),
                        in0=ps_xw[:],
                        scalar1=biasT_sb[:, c : c + 1],
                        scalar2=None,
                        op0=mybir.AluOpType.add,
                    )

            # ---- phase 1: LSTM recurrence ----
            with (
                tc.tile_pool(name="p1", bufs=3) as gp,
                tc.tile_pool(name="p1ps", bufs=2, space="PSUM") as psg,
            ):
                for t in range(T):
                    ps_g = psg.tile([128, NCH, BL], f32, tag="psg")
                    for c in range(NCH):
                        for k in range(2):
                            rhs = (
                                h_init[:, k, :]
                                if t == 0
                                else z01[:, k, :, t - 1]
                            )
                            nc.tensor.matmul(
                                ps_g[:, c, :],
                                whh_sb[:, k, c * 128 : (c + 1) * 128],
                                rhs,
                                start=(k == 0),
                                stop=(k == 1),
                            )
                    gates = gp.tile([128, NCH, BL], f32, tag="gates")
                    nc.vector.tensor_tensor(
                        out=gates[:], in0=ps_g[:], in1=xwT[:, :, t, :],
                        op=mybir.AluOpType.add,
                    )
                    nc.scalar.activation(
                        out=gates[:, 0:4], in_=gates[:, 0:4],
                        func=mybir.ActivationFunctionType.Sigmoid,
                    )
                    nc.scalar.activation(
                        out=gates[:, 4:6], in_=gates[:, 4:6],
                        func=mybir.ActivationFunctionType.Tanh,
                    )
                    nc.scalar.activation(
                        out=gates[:, 6:8], in_=gates[:, 6:8],
                        func=mybir.ActivationFunctionType.Sigmoid,
                    )
                    # c = sig(f)*c + sig(i)*tanh(g)
                    nc.vector.tensor_tensor(
                        out=cT[:], in0=gates[:, 2:4], in1=cT[:],
                        op=mybir.AluOpType.mult,
                    )
                    ig = gp.tile([128, 2, BL], f32, tag="ig")
                    nc.vector.tensor_tensor(
                        out=ig[:], in0=gates[:, 0:2], in1=gates[:, 4:6],
                        op=mybir.AluOpType.mult,
                    )
                    nc.vector.tensor_tensor(
                        out=cT[:], in0=cT[:], in1=ig[:], op=mybir.AluOpType.add
                    )
                    th = gp.tile([128, 2, BL], f32, tag="th")
                    nc.scalar.activation(
                        out=th[:], in_=cT[:], func=mybir.ActivationFunctionType.Tanh
                    )
                    # h = sig(o) * tanh(c)  -> straight into Z (bf16)
                    nc.vector.tensor_tensor(
                        out=z01[:, :, :, t], in0=gates[:, 6:8], in1=th[:],
                        op=mybir.AluOpType.mult,
                    )

            # ---- phase 2: attention over all timesteps ----
            with (
                tc.tile_pool(name="p2", bufs=2) as ap,
                tc.tile_pool(name="p2ps", bufs=2, space="PSUM") as ps2,
            ):
                expsc = pp.tile([S, BL, T], bf16)
                for b in range(BL):
                    ps_s = ps2.tile([S, T], f32, tag="ps_s")
                    for k in range(2):
                        nc.tensor.matmul(
                            ps_s[:],
                            encT_sb[:, k, b, :],
                            z01[:, k, b, :],
                            start=(k == 0),
                            stop=(k == 1),
                        )
                    nc.scalar.activation(
                        out=expsc[:, b, :], in_=ps_s[:],
                        func=mybir.ActivationFunctionType.Exp,
                        scale=float(1.0 / np.sqrt(H)),
                    )
                    nc.vector.tensor_scalar_mul(
                        out=expsc[:, b, :], in0=expsc[:, b, :],
                        scalar1=mask01[:, b : b + 1],
                    )
                    ps_d = ps2.tile([1, T], f32, tag="ps_d")
                    nc.tensor.matmul(
                        ps_d[:], ones_col[:], expsc[:, b, :], start=True, stop=True
                    )
                    recip = ap.tile([1, T], f32, tag="recip")
                    nc.vector.reciprocal(out=recip[:], in_=ps_d[:])
                    ps_bc = ps2.tile([128, T], f32, tag="ps_bc")
                    nc.tensor.matmul(
                        ps_bc[:], ones_row_f[:], recip[:], start=True, stop=True
                    )
                    bc_sb = ap.tile([128, T], f32, tag="bc")
                    nc.vector.tensor_copy(out=bc_sb[:], in_=ps_bc[:])
                    for j in range(2):
                        ps_c = ps2.tile([128, T], f32, tag="ps_c")
                        nc.tensor.matmul(
                            ps_c[:],
                            enc_sb[:, b, j * 128 : (j + 1) * 128],
                            expsc[:, b, :],
                            start=True,
                            stop=True,
                        )
                        nc.vector.tensor_tensor(
                            out=z23[:, j, b, :], in0=ps_c[:], in1=bc_sb[:],
                            op=mybir.AluOpType.mult,
                        )

            # ---- phase 3: logits = Z @ W_lin^T + b_lin ----
            zt = [
                z01[:, 0].rearrange("p b t -> p (b t)"),
                z01[:, 1].rearrange("p b t -> p (b t)"),
                z23[:, 0].rearrange("p b t -> p (b t)"),
                z23[:, 1].rearrange("p b t -> p (b t)"),
            ]
            with (
                tc.tile_pool(name="p3rhs", bufs=2) as rp,
                tc.tile_pool(name="p3out", bufs=3) as op_,
                tc.tile_pool(name="p3bl", bufs=2) as blp,
                tc.tile_pool(name="p3ps", bufs=2, space="PSUM") as ps3,
            ):
                for ng in range(NGN):
                    n0 = ng * NGC
                    rhs_t = rp.tile([128, 4, NGC], bf16, tag="rhs")
                    for k in range(4):
                        nc.sync.dma_start(
                            out=rhs_t[:, k, :], in_=wlinT_d[k][:, n0 : n0 + NGC]
                        )
                    bl_t = blp.tile([1, NGC], bf16, tag="blt")
                    nc.sync.dma_start(out=bl_t[:], in_=blin_d[:, n0 : n0 + NGC])
                    ps_bl = ps3.tile([128, 4, 512], f32, tag="po")
                    for n in range(4):
                        nc.tensor.matmul(
                            ps_bl[:, n, :NC_N],
                            ones_row_bf[:],
                            bl_t[:, n * NC_N : (n + 1) * NC_N],
                            start=True,
                            stop=True,
                        )
                    bl_sb = blp.tile([128, NGC], bf16, tag="blsb")
                    nc.scalar.copy(
                        out=bl_sb[:].rearrange("p (g n) -> p g n", g=4),
                        in_=ps_bl[:, :, :NC_N],
                    )
                    for m0, msz in M_TILES:
                        ps_o = ps3.tile([128, 4, 512], f32, tag="po")
                        for k in range(4):
                            for n in range(4):
                                nc.tensor.matmul(
                                    ps_o[:msz, n, :NC_N],
                                    zt[k][:, m0 : m0 + msz],
                                    rhs_t[:, k, n * NC_N : (n + 1) * NC_N],
                                    start=(k == 0),
                                    stop=(k == 3),
                                )
                        o_sb = op_.tile([128, NGC], f32, tag="osb")
                        nc.vector.tensor_tensor(
                            out=o_sb[:msz].rearrange("p (g n) -> p g n", g=4),
                            in0=ps_o[:msz, :, :NC_N],
                            in1=bl_sb[:msz].rearrange("p (g n) -> p g n", g=4),
                            op=mybir.AluOpType.add,
                        )
                        nc.sync.dma_start(
                            out=out_d[m0 : m0 + msz, n0 : n0 + NGC],
                            in_=o_sb[:msz],
                        )
    nc.compile()
    return nc


def _prep_inputs(inputs):
    bf = ml_dtypes.bfloat16
    target = np.asarray(inputs["target_tensor"])
    enc = np.asarray(inputs["encoder_outputs"], dtype=np.float32)
    lens = np.asarray(inputs["encoder_seq_lens"])
    h0 = np.asarray(inputs["h0"], dtype=np.float32)
    c0 = np.asarray(inputs["c0"], dtype=np.float32)
    emb = np.ascontiguousarray(np.asarray(inputs["emb"], dtype=np.float32))
    W_ih = np.asarray(inputs["W_ih"], dtype=np.float32)
    W_hh = np.asarray(inputs["W_hh"], dtype=np.float32)
    bias = (
        np.asarray(inputs["b_ih"], dtype=np.float32)
        + np.asarray(inputs["b_hh"], dtype=np.float32)
    )
    W_lin = np.asarray(inputs["W_lin"], dtype=np.float32)
    b_lin = np.asarray(inputs["b_lin"], dtype=np.float32)

    wihT = np.ascontiguousarray(W_ih.T.astype(bf))                # (E, 4H)
    whhT = np.ascontiguousarray(
        W_hh.T.reshape(2, 128, G4H).astype(bf)
    )                                                             # (2,128,4H)
    biasT = np.ascontiguousarray(bias.reshape(NCH, 128).T)        # (128, NCH)
    wlinT = np.ascontiguousarray(
        W_lin.T.reshape(4, 128, V).astype(bf)
    )                                                             # (4,128,V)
    blin = np.ascontiguousarray(b_lin.reshape(1, V).astype(bf))   # (1, V)

    in_maps = []
    for i in range(NCORES):
        sl = slice(i * BL, (i + 1) * BL)
        tok = np.ascontiguousarray(
            target[sl].T.reshape(BT, 1).astype(np.int32)
        )  # t-major
        enc_i = enc[sl]                                           # (BL, S, H)
        enc_sbh = np.ascontiguousarray(
            enc_i.transpose(1, 0, 2).astype(bf)
        )                                                         # (S, BL, H)
        encT = np.ascontiguousarray(
            enc_i.transpose(2, 0, 1).reshape(2, 128, BL, S).astype(bf)
        )                                                         # (2,128,BL,S)
        h0T = np.ascontiguousarray(h0[sl].T.reshape(2, 128, BL).transpose(1, 0, 2))
        c0T = np.ascontiguousarray(c0[sl].T.reshape(2, 128, BL).transpose(1, 0, 2))
        in_maps.append(
            {
                "tok": tok,
                "emb": emb,
                "enc": enc_sbh,
                "encT": encT,
                "h0T": h0T,
                "c0T": c0T,
                "lens": np.ascontiguousarray(lens[sl].astype(np.int32)),
                "biasT": biasT,
                "wihT": wihT,
                "whhT": whhT,
                "wlinT": wlinT,
                "blin": blin,
            }
        )
    return in_maps


LAST_RESULTS = None


def _install_ntff_shim():
    """Provide antenv.axon_hooks if the image's antenv lacks it, so
    trace=True/BASS_TRACE=1 can capture NTFF profiles under axon."""
    import sys
    import types

    try:
        from antenv.axon_hooks import get_axon_ntff_profile_hook  # noqa: F401

        return
    except ImportError:
        pass
    try:
        from trn_agent_boot.trn_boot import _ntff_profile_via_ctypes

        hook = _ntff_profile_via_ctypes("/opt/axon/libaxon_pjrt.so")
        m = types.ModuleType("antenv.axon_hooks")
        m.get_axon_ntff_profile_hook = lambda: hook
        m.set_axon_ntff_profile_hook = lambda h: None
        sys.modules["antenv.axon_hooks"] = m
    except Exception:
        pass


def kernel(**inputs):
    global LAST_RESULTS
    _install_ntff_shim()
    if "nc" not in _CACHE:
        _CACHE["nc"] = _build()
    nc = _CACHE["nc"]
    in_maps = _prep_inputs(inputs)
    res = run_bass_kernel_spmd(nc, in_maps, core_ids=list(range(NCORES)))
    LAST_RESULTS = res
    out = np.empty((B, T, V), dtype=np.float32)
    for i in range(NCORES):
        # logits rows = Z columns = b*T + t (b-major)
        out[i * BL : (i + 1) * BL] = res.results[i]["logits"].reshape(BL, T, V)
    return out


# revision 14
# speedup vs baseline: 1.0556x; 1.0556x over previous
"""Trainium2 Bass kernel for DecoderAttnRNN (LSTM + attention decoder).

Sharding: data-parallel over batch B=64 across 8 cores (8 batches/core).
Each core:
  phase 0: gather embeddings (indirect DMA), transpose, precompute x@W_ih.T+bias
  phase 1: 72-step LSTM recurrence in transposed layout (features on
           partitions, batch on free dim)
  phase 2: attention for all (b,t) at once (ctx does not feed the recurrence)
  phase 3: big output projection (576 x 512) @ (512 x 32000) in bf16,
           streaming W_lin^T from DRAM, b_lin added via K=1 broadcast matmuls
"""

import numpy as np
import ml_dtypes

import concourse.bass as bass
import concourse.mybir as mybir
import concourse.tile as tile
from concourse import bacc
from concourse.bass_utils import run_bass_kernel_spmd
from concourse.masks import make_identity

B, T, S, E, H, V = 64, 72, 72, 128, 256, 32000
NCORES = 8
BL = B // NCORES          # 8 batches per core
BT = BL * T               # 576 (t-major: flat index = t*BL + b)
G4H = 4 * H               # 1024
NCH = G4H // 128          # 8 gate chunks of 128
NC_N = 500                # psum n-chunk for phase 3 (4 per group)
NGC = 4 * NC_N            # 2000 cols per n-group
NGN = V // NGC            # 16 groups

f32 = mybir.dt.float32
bf16 = mybir.dt.bfloat16
i32 = mybir.dt.int32

M_TILES = [(0, 128), (128, 128), (256, 128), (384, 128), (512, 64)]

_CACHE = {}


def _build():
    nc = bacc.Bacc(None, target_bir_lowering=False)

    tok_d = nc.declare_dram_parameter("tok", [BT, 1], i32, isOutput=False)
    emb_d = nc.declare_dram_parameter("emb", [V, E], f32, isOutput=False)
    enc_d = nc.declare_dram_parameter("enc", [S, BL, H], bf16, isOutput=False)
    encT_d = nc.declare_dram_parameter("encT", [2, 128, BL, S], bf16, isOutput=False)
    h0T_d = nc.declare_dram_parameter("h0T", [128, 2, BL], f32, isOutput=False)
    c0T_d = nc.declare_dram_parameter("c0T", [128, 2, BL], f32, isOutput=False)
    lens_d = nc.declare_dram_parameter("lens", [BL], i32, isOutput=False)
    biasT_d = nc.declare_dram_parameter("biasT", [128, NCH], f32, isOutput=False)
    wihT_d = nc.declare_dram_parameter("wihT", [E, G4H], bf16, isOutput=False)
    whhT_d = nc.declare_dram_parameter("whhT", [2, 128, G4H], bf16, isOutput=False)
    wlinT_d = nc.declare_dram_parameter("wlinT", [4, 128, V], bf16, isOutput=False)
    blin_d = nc.declare_dram_parameter("blin", [1, V], bf16, isOutput=False)
    out_d = nc.declare_dram_parameter("logits", [BT, V], f32, isOutput=True)

    with tile.TileContext(nc) as tc:
        with tc.tile_pool(name="persist", bufs=1) as pp:
            # ---- persistent tiles ----
            wih_sb = pp.tile([128, G4H], bf16)
            nc.sync.dma_start(out=wih_sb[:], in_=wihT_d[:])
            whh_sb = pp.tile([128, 2, G4H], bf16)
            for k in range(2):
                nc.sync.dma_start(out=whh_sb[:, k, :], in_=whhT_d[k])
            biasT_sb = pp.tile([128, NCH], f32)
            nc.sync.dma_start(out=biasT_sb[:], in_=biasT_d[:])
            encT_sb = pp.tile([128, 2, BL, S], bf16)
            for k in range(2):
                nc.sync.dma_start(out=encT_sb[:, k], in_=encT_d[k])
            enc_sb = pp.tile([S, BL, H], bf16)
            nc.sync.dma_start(out=enc_sb[:], in_=enc_d[:])

            xwT = pp.tile([128, T, NCH, BL], f32)      # x@W_ih.T + bias, t-major
            z01 = pp.tile([128, 2, BL, T], bf16)       # h features (k-tiles 0,1)
            z23 = pp.tile([128, 2, BL, T], bf16)       # ctx features (k-tiles 2,3)
            x_allT = pp.tile([128, BT], bf16)          # embeddings^T, (t,b) cols

            cT = pp.tile([128, 2, BL], f32)
            nc.sync.dma_start(out=cT[:], in_=c0T_d[:])
            h0f = pp.tile([128, 2, BL], f32)
            nc.sync.dma_start(out=h0f[:], in_=h0T_d[:])
            h_init = pp.tile([128, 2, BL], bf16)
            nc.vector.tensor_copy(out=h_init[:], in_=h0f[:])

            ident = pp.tile([128, 128], f32)
            make_identity(nc, ident[:])
            ones_col = pp.tile([S, 1], bf16)
            nc.vector.memset(ones_col[:], 1.0)
            ones_row_f = pp.tile([1, 128], f32)
            nc.vector.memset(ones_row_f[:], 1.0)
            ones_row_bf = pp.tile([1, 128], bf16)
            nc.vector.memset(ones_row_bf[:], 1.0)

            # ---- attention mask: mask01[s, b] = 1.0 if s < len_b else 0.0 ----
            lens_i = pp.tile([S, BL], i32)
            lens_bcast = bass.AP(tensor=lens_d, offset=0, ap=[[0, S], [1, BL]])
            nc.sync.dma_start(out=lens_i[:], in_=lens_bcast)
            lens_f = pp.tile([S, BL], f32)
            nc.vector.tensor_copy(out=lens_f[:], in_=lens_i[:])
            iota_i = pp.tile([S, 1], i32)
            nc.gpsimd.iota(iota_i[:], [[1, 1]], base=0, channel_multiplier=1)
            iota_f = pp.tile([S, 1], f32)
            nc.vector.tensor_copy(out=iota_f[:], in_=iota_i[:])
            mask01 = pp.tile([S, BL], f32)
            nc.vector.tensor_scalar(
                out=mask01[:], in0=lens_f[:], scalar1=iota_f[:], scalar2=None,
                op0=mybir.AluOpType.is_gt,
            )

            # ---- phase 0: embedding gather + transpose ----
            with (
                tc.tile_pool(name="p0", bufs=2) as wp,
                tc.tile_pool(name="p0ps", bufs=2, space="PSUM") as psp,
            ):
                for j in range(5):
                    n = 128 if j < 4 else BT - 4 * 128
                    tok_t = wp.tile([128, 1], i32, tag="tok")
                    nc.sync.dma_start(
                        out=tok_t[:n], in_=tok_d[j * 128 : j * 128 + n]
                    )
                    x_t = wp.tile([128, E], f32, tag="x")
                    nc.gpsimd.indirect_dma_start(
                        out=x_t[:n],
                        out_offset=None,
                        in_=emb_d[:],
                        in_offset=bass.IndirectOffsetOnAxis(ap=tok_t[:n, :1], axis=0),
                    )
                    ps_t = psp.tile([128, 128], f32, tag="pst")
                    nc.tensor.transpose(
                        out=ps_t[:, :n], in_=x_t[:n], identity=ident[:n, :n]
                    )
                    nc.vector.tensor_copy(
                        out=x_allT[:, j * 128 : j * 128 + n], in_=ps_t[:, :n]
                    )

                # xW precompute: xwT[:, c, t, b] = (x @ W_ih.T)[tb, c*128:...] + bias
                for c in range(NCH):
                    ps_xw = psp.tile([128, BT], f32, tag="psxw")
                    for n0, nn in [(0, 512), (512, BT - 512)]:
                        nc.tensor.matmul(
                            ps_xw[:, n0 : n0 + nn],
                            wih_sb[:, c * 128 : (c + 1) * 128],
                            x_allT[:, n0 : n0 + nn],
                            start=True,
                            stop=True,
                        )
                    nc.vector.tensor_scalar(
                        out=xwT[:, :, c, :],
                        in0=ps_xw[:].rearrange("p (t b) -> p t b", b=BL),
                        scalar1=biasT_sb[:, c : c + 1],
                        scalar2=None,
                        op0=mybir.AluOpType.add,
                    )

            # ---- phase 1: LSTM recurrence ----
            with (
                tc.tile_pool(name="p1", bufs=3) as gp,
                tc.tile_pool(name="p1ps", bufs=2, space="PSUM") as psg,
            ):
                for t in range(T):
                    ps_g = psg.tile([128, NCH, BL], f32, tag="psg")
                    for c in range(NCH):
                        for k in range(2):
                            rhs = (
                                h_init[:, k, :]
                                if t == 0
                                else z01[:, k, :, t - 1]
                            )
                            nc.tensor.matmul(
                                ps_g[:, c, :],
                                whh_sb[:, k, c * 128 : (c + 1) * 128],
                                rhs,
                                start=(k == 0),
                                stop=(k == 1),
                            )
                    # gate order is host-permuted to (i, f, o, g):
                    # chunks 0-1=i, 2-3=f, 4-5=o, 6-7=g
                    gates = gp.tile([128, NCH, BL], f32, tag="gates")
                    nc.vector.tensor_tensor(
                        out=gates[:], in0=ps_g[:], in1=xwT[:, t],
                        op=mybir.AluOpType.add,
                    )
                    nc.scalar.activation(
                        out=gates[:, 6:8], in_=gates[:, 6:8],
                        func=mybir.ActivationFunctionType.Tanh,
                    )
                    nc.scalar.activation(
                        out=gates[:, 0:6], in_=gates[:, 0:6],
                        func=mybir.ActivationFunctionType.Sigmoid,
                    )
                    # c = sig(f)*c + sig(i)*tanh(g)
                    nc.vector.tensor_tensor(
                        out=cT[:], in0=gates[:, 2:4], in1=cT[:],
                        op=mybir.AluOpType.mult,
                    )
                    ig = gp.tile([128, 2, BL], f32, tag="ig")
                    nc.vector.tensor_tensor(
                        out=ig[:], in0=gates[:, 0:2], in1=gates[:, 6:8],
                        op=mybir.AluOpType.mult,
                    )
                    nc.vector.tensor_tensor(
                        out=cT[:], in0=cT[:], in1=ig[:], op=mybir.AluOpType.add
                    )
                    th = gp.tile([128, 2, BL], f32, tag="th")
                    nc.scalar.activation(
                        out=th[:], in_=cT[:], func=mybir.ActivationFunctionType.Tanh
                    )
                    # h = sig(o) * tanh(c)  -> straight into Z (bf16)
                    nc.vector.tensor_tensor(
                        out=z01[:, :, :, t], in0=gates[:, 4:6], in1=th[:],
                        op=mybir.AluOpType.mult,
                    )

            # ---- phase 2: attention over all timesteps ----
            with (
                tc.tile_pool(name="p2", bufs=2) as ap,
                tc.tile_pool(name="p2ps", bufs=2, space="PSUM") as ps2,
            ):
                expsc = pp.tile([S, BL, T], bf16)
                for b in range(BL):
                    ps_s = ps2.tile([S, T], f32, tag="ps_s")
                    for k in range(2):
                        nc.tensor.matmul(
                            ps_s[:],
                            encT_sb[:, k, b, :],
                            z01[:, k, b, :],
                            start=(k == 0),
                            stop=(k == 1),
                        )
                    nc.scalar.activation(
                        out=expsc[:, b, :], in_=ps_s[:],
                        func=mybir.ActivationFunctionType.Exp,
                        scale=float(1.0 / np.sqrt(H)),
                    )
                    nc.vector.tensor_scalar_mul(
                        out=expsc[:, b, :], in0=expsc[:, b, :],
                        scalar1=mask01[:, b : b + 1],
                    )
                    ps_d = ps2.tile([1, T], f32, tag="ps_d")
                    nc.tensor.matmul(
                        ps_d[:], ones_col[:], expsc[:, b, :], start=True, stop=True
                    )
                    recip = ap.tile([1, T], f32, tag="recip")
                    nc.vector.reciprocal(out=recip[:], in_=ps_d[:])
                    ps_bc = ps2.tile([128, T], f32, tag="ps_bc")
                    nc.tensor.matmul(
                        ps_bc[:], ones_row_f[:], recip[:], start=True, stop=True
                    )
                    bc_sb = ap.tile([128, T], f32, tag="bc")
                    nc.vector.tensor_copy(out=bc_sb[:], in_=ps_bc[:])
                    for j in range(2):
                        ps_c = ps2.tile([128, T], f32, tag="ps_c")
                        nc.tensor.matmul(
                            ps_c[:],
                            enc_sb[:, b, j * 128 : (j + 1) * 128],
                            expsc[:, b, :],
                            start=True,
                            stop=True,
                        )
                        nc.vector.tensor_tensor(
                            out=z23[:, j, b, :], in0=ps_c[:], in1=bc_sb[:],
                            op=mybir.AluOpType.mult,
                        )

            # ---- phase 3: logits = Z @ W_lin^T + b_lin ----
            zt = [
                z01[:, 0].rearrange("p b t -> p (b t)"),
                z01[:, 1].rearrange("p b t -> p (b t)"),
                z23[:, 0].rearrange("p b t -> p (b t)"),
                z23[:, 1].rearrange("p b t -> p (b t)"),
            ]
            with (
                tc.tile_pool(name="p3rhs", bufs=3) as rp,
                tc.tile_pool(name="p3out", bufs=4) as op_,
                tc.tile_pool(name="p3bl", bufs=2) as blp,
                tc.tile_pool(name="p3ps", bufs=2, space="PSUM") as ps3,
            ):
                for ng in range(NGN):
                    n0 = ng * NGC
                    rhs_t = rp.tile([128, 4, NGC], bf16, tag="rhs")
                    for k in range(4):
                        nc.sync.dma_start(
                            out=rhs_t[:, k, :], in_=wlinT_d[k][:, n0 : n0 + NGC]
                        )
                    # b_lin broadcast to all partitions via stride-0 DMA
                    bl_sb = blp.tile([128, NGC], bf16, tag="blsb")
                    bl_bcast = bass.AP(
                        tensor=blin_d, offset=n0, ap=[[0, 128], [1, NGC]]
                    )
                    nc.scalar.dma_start(out=bl_sb[:], in_=bl_bcast)
                    for m0, msz in M_TILES:
                        ps_o = ps3.tile([128, 4, 512], f32, tag="po")
                        for k in range(4):
                            for n in range(4):
                                nc.tensor.matmul(
                                    ps_o[:msz, n, :NC_N],
                                    zt[k][:, m0 : m0 + msz],
                                    rhs_t[:, k, n * NC_N : (n + 1) * NC_N],
                                    start=(k == 0),
                                    stop=(k == 3),
                                )
                        o_sb = op_.tile([128, NGC], f32, tag="osb")
                        nc.vector.tensor_tensor(
                            out=o_sb[:msz].rearrange("p (g n) -> p g n", g=4),
                            in0=ps_o[:msz, :, :NC_N],
                            in1=bl_sb[:msz].rearrange("p (g n) -> p g n", g=4),
                            op=mybir.AluOpType.add,
                        )
                        nc.gpsimd.dma_start(
                            out=out_d[m0 : m0 + msz, n0 : n0 + NGC],
                            in_=o_sb[:msz],
                        )
    nc.compile()
    return nc


def _prep_inputs(inputs):
    bf = ml_dtypes.bfloat16
    target = np.asarray(inputs["target_tensor"])
    enc = np.asarray(inputs["encoder_outputs"], dtype=np.float32)
    lens = np.asarray(inputs["encoder_seq_lens"])
    h0 = np.asarray(inputs["h0"], dtype=np.float32)
    c0 = np.asarray(inputs["c0"], dtype=np.float32)
    emb = np.ascontiguousarray(np.asarray(inputs["emb"], dtype=np.float32))
    W_ih = np.asarray(inputs["W_ih"], dtype=np.float32)
    W_hh = np.asarray(inputs["W_hh"], dtype=np.float32)
    bias = (
        np.asarray(inputs["b_ih"], dtype=np.float32)
        + np.asarray(inputs["b_hh"], dtype=np.float32)
    )
    # permute gate order (i, f, g, o) -> (i, f, o, g) so the device can run
    # one sigmoid over the first 6 chunks and one tanh over the last 2
    perm = np.concatenate(
        [np.arange(0, 2 * H), np.arange(3 * H, 4 * H), np.arange(2 * H, 3 * H)]
    )
    W_ih = W_ih[perm]
    W_hh = W_hh[perm]
    bias = bias[perm]
    W_lin = np.asarray(inputs["W_lin"], dtype=np.float32)
    b_lin = np.asarray(inputs["b_lin"], dtype=np.float32)

    wihT = np.ascontiguousarray(W_ih.T.astype(bf))                # (E, 4H)
    whhT = np.ascontiguousarray(
        W_hh.T.reshape(2, 128, G4H).astype(bf)
    )                                                             # (2,128,4H)
    biasT = np.ascontiguousarray(bias.reshape(NCH, 128).T)        # (128, NCH)
    wlinT = np.ascontiguousarray(
        W_lin.T.reshape(4, 128, V).astype(bf)
    )                                                             # (4,128,V)
    blin = np.ascontiguousarray(b_lin.reshape(1, V).astype(bf))   # (1, V)

    in_maps = []
    for i in range(NCORES):
        sl = slice(i * BL, (i + 1) * BL)
        tok = np.ascontiguousarray(
            target[sl].T.reshape(BT, 1).astype(np.int32)
        )  # t-major
        enc_i = enc[sl]                                           # (BL, S, H)
        enc_sbh = np.ascontiguousarray(
            enc_i.transpose(1, 0, 2).astype(bf)
        )                                                         # (S, BL, H)
        encT = np.ascontiguousarray(
            enc_i.transpose(2, 0, 1).reshape(2, 128, BL, S).astype(bf)
        )                                                         # (2,128,BL,S)
        h0T = np.ascontiguousarray(h0[sl].T.reshape(2, 128, BL).transpose(1, 0, 2))
        c0T = np.ascontiguousarray(c0[sl].T.reshape(2, 128, BL).transpose(1, 0, 2))
        in_maps.append(
            {
                "tok": tok,
                "emb": emb,
                "enc": enc_sbh,
                "encT": encT,
                "h0T": h0T,
                "c0T": c0T,
                "lens": np.ascontiguousarray(lens[sl].astype(np.int32)),
                "biasT": biasT,
                "wihT": wihT,
                "whhT": whhT,
                "wlinT": wlinT,
                "blin": blin,
            }
        )
    return in_maps


LAST_RESULTS = None


def _install_ntff_shim():
    """Provide antenv.axon_hooks if the image's antenv lacks it, so
    trace=True/BASS_TRACE=1 can capture NTFF profiles under axon."""
    import sys
    import types

    try:
        from antenv.axon_hooks import get_axon_ntff_profile_hook  # noqa: F401

        return
    except ImportError:
        pass
    try:
        from trn_agent_boot.trn_boot import _ntff_profile_via_ctypes

        hook = _ntff_profile_via_ctypes("/opt/axon/libaxon_pjrt.so")
        m = types.ModuleType("antenv.axon_hooks")
        m.get_axon_ntff_profile_hook = lambda: hook
        m.set_axon_ntff_profile_hook = lambda h: None
        sys.modules["antenv.axon_hooks"] = m
    except Exception:
        pass


def kernel(**inputs):
    global LAST_RESULTS
    _install_ntff_shim()
    if "nc" not in _CACHE:
        _CACHE["nc"] = _build()
    nc = _CACHE["nc"]
    in_maps = _prep_inputs(inputs)
    res = run_bass_kernel_spmd(nc, in_maps, core_ids=list(range(NCORES)))
    LAST_RESULTS = res
    out = np.empty((B, T, V), dtype=np.float32)
    for i in range(NCORES):
        # logits rows = Z columns = b*T + t (b-major)
        out[i * BL : (i + 1) * BL] = res.results[i]["logits"].reshape(BL, T, V)
    return out
